# revision 4
# baseline (speedup 1.0000x reference)
"""Causal multi-head attention block (B=4, S=2048, NX=1024, H=16, D=64)
distributed over 8 TRN2 NeuronCores.

Sharding: core i handles batch b = i//2 and head-group hg = i%2 (8 of 16
heads).  Each core computes qkv for its heads, causal attention, and a
partial c_proj over its 512 feature rows; the per-batch pair of partials
is summed on the host while unsharding.

All matmuls run in bf16 (f32 PSUM accumulate).  Scores are computed in the
transposed orientation s^T[k, q] = k @ q^T.  The u = p @ v stage also runs
transposed: stationary = v_aug[k, (v | ones)] (ones half parity-swapped per
head), moving = the exp'd score tile p[k, q], accumulating u^T[d, q] in
PSUM with the softmax denominator replicated on the other 64 partitions.
This cuts the u matmuls to one wide-moving MM per (head, k-tile) with no
LDWEIGHTS churn, and writes a^T directly (no PE transpose pass).
finalize = DMA partition-shift of the denominator + reciprocal + one
tensor_tensor multiply per (head, q-chunk).
"""
import sys

sys.path.insert(0, "/opt/trn_rl_repo")

import functools

import ml_dtypes
import numpy as np

from concourse import bacc, mybir, tile
from concourse.bass_utils import run_bass_kernel_spmd

B, S, NX = 4, 2048, 1024
H, D = 16, 64
N_CORES = 8
HL = H // 2          # heads per core (local)
FL = HL * D          # local head feature width (512)
BF16 = mybir.dt.bfloat16
F32 = mybir.dt.float32
BF = ml_dtypes.bfloat16

NK = S // 128        # 16 k-tiles of 128
NQC = S // 512       # 4 q-chunks of 512
KK = NX // 128       # 8 contraction blocks

DEFAULT_CFG = "host-psw1536-psb2-pb8-pub2-nb-sc-xp"
DEFAULT_CFG_BIAS = "host-psw1536-psb2-pb8-pub2-sc-xp"


def _parse_cfg(cfg: str):
    parts = cfg.split("-")
    d = {"mode": parts[0], "psw": 1536, "psb": 2, "pb": 8, "pub": 2,
         "nb": False, "sc": False, "do": False, "xp": False, "gm": False,
         "ac": False}
    for p in parts[1:]:
        if p.startswith("psw"):
            d["psw"] = int(p[3:])
        elif p.startswith("psb"):
            d["psb"] = int(p[3:])
        elif p.startswith("pub"):
            d["pub"] = int(p[3:])
        elif p.startswith("pb"):
            d["pb"] = int(p[2:])
        elif p in d:
            d[p] = True
    return d


def _build(cfg: str):
    c = _parse_cfg(cfg)
    PSW, PSB, PB, PUB = c["psw"], c["psb"], c["pb"], c["pub"]
    NB, SC, DO, XP, GM = c["nb"], c["sc"], c["do"], c["xp"], c["gm"]
    GK = PSW // 512   # full k-tiles per exp group
    nc = bacc.Bacc("TRN2", target_bir_lowering=False, debug=False,
                   num_devices=N_CORES)

    xT_ext = nc.dram_tensor("xT", [NX, S], BF16, kind="ExternalInput")
    wqk_ext = nc.dram_tensor("w_qk", [NX, 2 * FL], BF16, kind="ExternalInput")
    wv_ext = nc.dram_tensor("w_v", [NX, FL], BF16, kind="ExternalInput")
    wp_ext = nc.dram_tensor("w_proj", [FL, NX], BF16, kind="ExternalInput")
    bqk_ext = nc.dram_tensor("b_qk", [2 * FL, 1], F32, kind="ExternalInput")
    bv_ext = nc.dram_tensor("bv_row", [1, FL], BF16, kind="ExternalInput")
    bp_ext = nc.dram_tensor("bp_row", [1, NX], BF16, kind="ExternalInput")
    out_ext = nc.dram_tensor("out", [S, NX], F32, kind="ExternalOutput")

    with tile.TileContext(nc) as tc:
        with tc.tile_pool(name="const", bufs=1) as cp, \
             tc.tile_pool(name="work", bufs=3) as wp, \
             tc.tile_pool(name="psS", bufs=PSB, space="PSUM") as psS, \
             tc.tile_pool(name="psU", bufs=PUB, space="PSUM") as psU:

            # ---- persistent SBUF tensors ----
            xT = cp.tile([128, KK, S], BF16, tag="xT")
            wqk = cp.tile([128, KK, 2 * FL], BF16, tag="wqk")
            wv = cp.tile([128, KK, FL], BF16, tag="wv")
            wproj = cp.tile([128, FL // 128, NX], BF16, tag="wproj")
            qkT = cp.tile([128, 2 * FL // 128, S], BF16, tag="qkT")
            # v_aug[k, kt, hh, par, 0:128]: per head pair hh, parity par:
            #   par=0 (even head): cols 0:64 = v, 64:128 = 1.0
            #   par=1 (odd head):  cols 0:64 = 1.0, 64:128 = v
            v5 = cp.tile([128, NK, HL // 2, 2, 128], BF16, tag="v5")
            aT = cp.tile([128, FL // 128, S], BF16, tag="aT")  # a^T [feat, q]
            bqk = cp.tile([128, 2 * FL // 128], F32, tag="bqk")
            bv_row = cp.tile([1, FL], BF16, tag="bv")
            bp_row = cp.tile([1, NX], BF16, tag="bp")
            ones_row = cp.tile([1, 128], BF16, tag="ones")
            tri = cp.tile([128, 128], BF16, tag="tri")

            # ---- input DMAs (ordered so compute can start early) ----
            for kk in range(KK):
                nc.sync.dma_start(out=wv[:, kk, :],
                                  in_=wv_ext.ap()[kk * 128:(kk + 1) * 128, :])
            # x chunked by S so v/qk matmuls start after the first chunk
            for sc in range(4):
                for kk in range(KK):
                    nc.sync.dma_start(
                        out=xT[:, kk, sc * 512:(sc + 1) * 512],
                        in_=xT_ext.ap()[kk * 128:(kk + 1) * 128,
                                        sc * 512:(sc + 1) * 512])
                if sc == 0:
                    for kk in range(KK):
                        nc.sync.dma_start(
                            out=wqk[:, kk, :],
                            in_=wqk_ext.ap()[kk * 128:(kk + 1) * 128, :])
            for kt in range(FL // 128):
                nc.sync.dma_start(out=wproj[:, kt, :],
                                  in_=wp_ext.ap()[kt * 128:(kt + 1) * 128, :])
            for fb in range(2 * FL // 128):
                nc.sync.dma_start(out=bqk[:, fb:fb + 1],
                                  in_=bqk_ext.ap()[fb * 128:(fb + 1) * 128, :])
            nc.sync.dma_start(out=bv_row[:], in_=bv_ext.ap())
            nc.sync.dma_start(out=bp_row[:], in_=bp_ext.ap())

            nc.vector.memset(ones_row[:], 1.0)
            # tri[p, f] = 1 if p <= f else 0 (keep-in on p > f, else fill 1)
            nc.vector.memset(tri[:], 0.0)
            nc.gpsimd.affine_select(
                out=tri[:], in_=tri[:],
                compare_op=mybir.AluOpType.is_gt,
                fill=1.0, base=0, pattern=[[-1, 128]], channel_multiplier=1,
            )
            gm_zero = nc.gpsimd.to_reg(0.0) if GM else None
            # ones halves of v_aug (parity-swapped)
            nc.vector.memset(v5[:, :, :, 0, 64:128], 1.0)
            nc.vector.memset(v5[:, :, :, 1, 0:64], 1.0)

            # ---- stage 2: v (natural layout, split by head parity) ----
            def emit_v(st):
                ps = psS.tile([128, FL], F32, tag="ps")
                for kk in range(KK):
                    nc.tensor.matmul(ps[:], xT[:, kk, st * 128:(st + 1) * 128],
                                     wv[:, kk, :], start=(kk == 0),
                                     stop=(NB and kk == KK - 1))
                if not NB:
                    nc.tensor.matmul(ps[:], ones_row[:], bv_row[:],
                                     start=False, stop=True)
                ps_r = ps[:].rearrange("p (hh par d) -> p hh par d",
                                       par=2, d=D)
                nc.vector.tensor_copy(v5[:, st, :, 0, 0:D], ps_r[:, :, 0, :])
                nc.vector.tensor_copy(v5[:, st, :, 1, D:128], ps_r[:, :, 1, :])

            # ---- stage 1: q^T / k^T (feature-major) ----
            def emit_qk(fb):
                qk_groups = ((0, 1536), (1536, 512)) if PSW >= 1536 else \
                            ((0, 1024), (1024, 1024))
                for n0, nw in qk_groups:
                    ps = psS.tile([128, nw], F32, tag="ps")
                    for c0 in range(0, nw, 512):
                        for kk in range(KK):
                            nc.tensor.matmul(
                                ps[:, c0:c0 + 512],
                                wqk[:, kk, fb * 128:(fb + 1) * 128],
                                xT[:, kk, n0 + c0:n0 + c0 + 512],
                                start=(kk == 0), stop=(kk == KK - 1))
                    if SC:
                        for s0 in range(0, nw, 512):
                            nc.vector.tensor_scalar_add(
                                qkT[:, fb, n0 + s0:n0 + s0 + 512],
                                ps[:, s0:s0 + 512], bqk[:, fb:fb + 1])
                    else:
                        nc.vector.tensor_scalar_add(qkT[:, fb, n0:n0 + nw],
                                                    ps[:], bqk[:, fb:fb + 1])

            # ---- stage 3: attention ----
            def head_groups(lh, qc):
                """Per-(head, q-chunk) closures: (emit_scores, emit_u) per
                k-tile group, plus finalize."""
                fbq = lh // 2
                fbk = FL // 128 + lh // 2
                po = (lh % 2) * 64
                qb = qc * 512
                n_full = 4 * qc
                groups = []
                kt0 = 0
                while kt0 < n_full:
                    g = min(GK, n_full - kt0)
                    groups.append([(kt0 + j, j * 512, 512, 0) for j in range(g)])
                    kt0 += g
                if PSW >= 1536:
                    diag_offs = (0, 512, 1024, 1280)
                    groups.append([(n_full + j, diag_offs[j], 512 - 128 * j,
                                    128 * j) for j in range(4)])
                else:
                    groups.append([(n_full + 0, 0, 512, 0),
                                   (n_full + 1, 512, 384, 128)])
                    groups.append([(n_full + 2, 0, 256, 256),
                                   (n_full + 3, 256, 128, 384)])

                state = {"pu": None}
                p_tiles = [None] * len(groups)
                last_kt = n_full + 3

                def mk_scores(gi, g):
                    def emit():
                        gw = max(off + N for (_, off, N, _) in g)
                        ps = psS.tile([128, PSW], F32, tag="ps")
                        p = wp.tile([128, PSW], BF16, tag="p", bufs=PB)
                        for (kt, off, N, qoff) in g:
                            nc.tensor.matmul(
                                ps[:, off:off + N],
                                qkT[po:po + 64, fbk, kt * 128:(kt + 1) * 128],
                                qkT[po:po + 64, fbq, qb + qoff:qb + 512],
                                start=True, stop=True)
                        nc.scalar.activation(p[:, 0:gw], ps[:, 0:gw],
                                             mybir.ActivationFunctionType.Exp,
                                             scale=0.125)
                        if g[0][0] >= n_full:
                            for (kt, off, N, qoff) in g:
                                if GM:
                                    nc.gpsimd.affine_select(
                                        out=p[:, off:off + 128],
                                        in_=p[:, off:off + 128],
                                        compare_op=mybir.AluOpType.is_le,
                                        fill=gm_zero, base=0,
                                        pattern=[[-1, 128]],
                                        channel_multiplier=1)
                                else:
                                    nc.vector.tensor_mul(p[:, off:off + 128],
                                                         p[:, off:off + 128],
                                                         tri[:])
                        p_tiles[gi] = p
                    return emit

                def mk_u(gi, g):
                    def emit():
                        if state["pu"] is None:
                            state["pu"] = psU.tile([128, 512], F32, tag="pu",
                                                   name="pu_t")
                        pu = state["pu"]
                        p = p_tiles[gi]
                        for (kt, off, N, qoff) in g:
                            nc.tensor.matmul(
                                pu[:, qoff:qoff + N],
                                v5[:, kt, lh >> 1, lh & 1, :],
                                p[:, off:off + N],
                                start=(kt == 0), stop=(kt == last_kt),
                                skip_group_check=True)
                    return emit

                def finalize():
                    pu = state["pu"]
                    ub = po                  # u partitions (parity layout)
                    db = 64 - po             # denominator partitions
                    rec = wp.tile([128, 512], F32, tag="rec", bufs=3)
                    if XP:
                        nc.vector.reciprocal(rec[db:db + 64, :],
                                             pu[db:db + 64, :])
                        nc.vector.tensor_mul(aT[po:po + 64, fbq, qb:qb + 512],
                                             pu[ub:ub + 64, :],
                                             rec[db:db + 64, :])
                    else:
                        den = wp.tile([128, 512], F32, tag="den", bufs=3)
                        nc.sync.dma_start(out=den[po:po + 64, :],
                                          in_=pu[db:db + 64, :])
                        nc.vector.reciprocal(rec[po:po + 64, :],
                                             den[po:po + 64, :])
                        nc.vector.tensor_mul(aT[po:po + 64, fbq, qb:qb + 512],
                                             pu[ub:ub + 64, :],
                                             rec[po:po + 64, :])

                return ([(mk_scores(gi, g), mk_u(gi, g))
                         for gi, g in enumerate(groups)], finalize)

            def emit_pair(lhA, lhB, qc):
                SA, finA = head_groups(lhA, qc)
                SB, finB = head_groups(lhB, qc)
                n = len(SA)
                SA[0][0]()
                SB[0][0]()
                for i in range(n):
                    if i + 1 < n:
                        SA[i + 1][0]()
                        SB[i + 1][0]()
                    SA[i][1]()
                    SB[i][1]()
                finA()
                finB()

            # ---- stage 4: c_proj partial from a^T ----
            def emit_proj(st):
                for n0 in range(0, NX, 512):
                    ps = psU.tile([128, 512], F32, tag="pu")
                    for kt in range(FL // 128):
                        nc.tensor.matmul(ps[:], aT[:, kt, st * 128:(st + 1) * 128],
                                         wproj[:, kt, n0:n0 + 512],
                                         start=(kt == 0),
                                         stop=(NB and kt == FL // 128 - 1))
                    if not NB:
                        nc.tensor.matmul(ps[:], ones_row[:],
                                         bp_row[:, n0:n0 + 512],
                                         start=False, stop=True)
                    dst = out_ext.ap()[st * 128:(st + 1) * 128, n0:n0 + 512]
                    if DO:
                        nc.sync.dma_start(out=dst, in_=ps[:])
                    else:
                        osb = wp.tile([128, 512], F32, tag="osb")
                        nc.vector.tensor_copy(osb[:], ps[:])
                        nc.sync.dma_start(out=dst, in_=osb[:])

            # ---- emission schedule ----
            for st in range(4):
                emit_v(st)
            emit_qk(0)
            emit_qk(FL // 128)
            emit_pair(0, 1, 0)
            for pr in range(1, HL // 2):
                for st in range(4 * pr, 4 * pr + 4):
                    emit_v(st)
                emit_qk(pr)
                emit_qk(FL // 128 + pr)
                emit_pair(2 * pr, 2 * pr + 1, 0)
            for qc in range(1, NQC):
                for pr in range(HL // 2):
                    emit_pair(2 * pr, 2 * pr + 1, qc)
                    emit_proj(4 * (qc - 1) + pr)
            for st in range(4 * (NQC - 1), NK):
                emit_proj(st)

    nc.compile()
    return nc


@functools.lru_cache(maxsize=2)
def _built(cfg: str):
    return _build(cfg)


def _in_maps(x, c_attn_w, c_attn_b, c_proj_w, c_proj_b):
    maps = []
    for core in range(N_CORES):
        b, hg = core // 2, core % 2
        f0 = hg * FL
        w_q = c_attn_w[:, f0:f0 + FL]
        w_k = c_attn_w[:, NX + f0:NX + f0 + FL]
        w_v = c_attn_w[:, 2 * NX + f0:2 * NX + f0 + FL]
        b_q = c_attn_b[f0:f0 + FL]
        b_k = c_attn_b[NX + f0:NX + f0 + FL]
        b_v = c_attn_b[2 * NX + f0:2 * NX + f0 + FL]
        maps.append({
            "xT": np.ascontiguousarray(x[b].T).astype(BF),
            "w_qk": np.concatenate([w_q, w_k], axis=1).astype(BF),
            "w_v": np.ascontiguousarray(w_v).astype(BF),
            "w_proj": np.ascontiguousarray(c_proj_w[f0:f0 + FL, :]).astype(BF),
            "b_qk": np.concatenate([b_q, b_k]).astype(np.float32).reshape(-1, 1),
            "bv_row": b_v.astype(BF).reshape(1, FL),
            "bp_row": (c_proj_b / 2.0).astype(BF).reshape(1, NX),
        })
    return maps


def _run(inputs, cfg=None, trace=False):
    if cfg is None:
        zero_bias = (not inputs["c_attn_b"].any()) and \
                    (not inputs["c_proj_b"].any())
        cfg = DEFAULT_CFG if zero_bias else DEFAULT_CFG_BIAS
    nc = _built(cfg)
    maps = _in_maps(inputs["x"], inputs["c_attn_w"], inputs["c_attn_b"],
                    inputs["c_proj_w"], inputs["c_proj_b"])
    res = run_bass_kernel_spmd(nc, maps, core_ids=list(range(N_CORES)),
                               trace=trace)
    out = np.empty((B, S, NX), dtype=np.float32)
    for b in range(B):
        out[b] = res.results[2 * b]["out"] + res.results[2 * b + 1]["out"]
    return out, res


def kernel(**inputs):
    out, _ = _run({k: np.asarray(v) for k, v in inputs.items()})
    return out
